# revision 17
# baseline (speedup 1.0000x reference)
"""Multi-head attention (B=2, T=2048, C=1024, H=16, hd=64, RoPE, full mask)
on 8 TRN2 NeuronCores.

Sharding: tensor-parallel over (batch, head-group). Core c handles batch
c//4 and heads [4*(c%4) .. 4*(c%4)+3]. Each core computes the QKV
projection for its 4 heads, full attention over T=2048, and a partial
output projection y = o_heads @ w_proj[:, cols].T. The host sums the 4
partial y's per batch (the tensor-parallel unshard reduction).

On-chip layout is "transposed everything" so attention needs no on-chip
transposes of the big tensors:
  - qT/kT stored [head_dim, T] (projection computed as w @ x.T)
  - scores computed directly transposed: sT[tk, tq] = k[tk] . q[tq]
  - softmax denominator via an appended ones-column on V (M=66 matmul)
  - o.T scaled by 1/den via full-width DVE reciprocal + one 0/1
    broadcast matmul (ebc)
RoPE uses full-width elementwise ops after a host-side row permutation
of w_q/w_k plus a DVE stream_shuffle; stream_shuffle permutes within
32-partition blocks, so rows are laid [e0:16, o0:16, e16:32, o16:32]
putting the rotation partner at p^16.

All inputs are pre-packed on the HOST into the exact SBUF partition-major
layout, so every DMA is a contiguous 2D copy with 4-8KB lines (3D
scatter patterns are descriptor-bound and 5-10x slower).

Schedule: the scalar-engine exp over the T^2 scores (~142us/core) is the
binding resource. The program is ONE globally software-pipelined tick
stream over all 128 (qb, pair, kb) ticks with the scores matmul running
one tick AHEAD of the exp and AV one tick behind:
    tick j:  [pump fillers] scores(j) | exp(j-1) | AV(j-2)
so filler work pumped between ticks can never stall the exp stream.
Everything outside the prologue (k(n0), q(qb0)) is a filler unit with a
deadline: remaining qkv projections, RoPE, per-pair finalize chains, and
output-projection chunks. Priority-ordered DMAs + PE/ACT warmup shrink
the head; the per-pair finalize and ACT-assisted output copies shrink
the tail.

Precision: f16 operands with fp32 PSUM accumulation everywhere.
"""

import heapq

import ml_dtypes  # noqa: F401
import numpy as np

import concourse.bacc as bacc
import concourse.mybir as mybir
import concourse.tile as tile
from concourse.bass_utils import run_bass_kernel_spmd

# Problem constants (hardcoded per contract)
B, T, C = 2, 2048, 1024
N_HEAD = 16
HD = 64
N_CORES = 8
HPC = 4  # heads per core
GC = HPC * HD  # head channels per core = 256

P = 128
KC = C // P  # 8 contraction chunks for the projections
NQB = 4  # query blocks
TQ = T // NQB  # 512
NKB = T // P  # 16 key blocks
VW = HD + 2  # 66: v + ones col + pad col
NT = 4 * 32  # total ticks

F32 = mybir.dt.float32
F32R = mybir.dt.float32r
F16 = mybir.dt.float16

_PROGRAM = None


def _build_program():
    nc = bacc.Bacc(
        "TRN2", target_bir_lowering=False, debug=False, num_devices=N_CORES
    )

    # all host-prepacked to SBUF layout: plain contiguous 2D DMAs
    xb_d = nc.dram_tensor("xb", [P, KC * T], F16, kind="ExternalInput").ap()
    wqkk_d = nc.dram_tensor("wqkk", [P, KC * 2 * P], F16, kind="ExternalInput").ap()
    wqkq_d = nc.dram_tensor("wqkq", [P, KC * 2 * P], F16, kind="ExternalInput").ap()
    wvp_d = nc.dram_tensor("wvp", [P, KC * GC], F16, kind="ExternalInput").ap()
    wpp_d = nc.dram_tensor("wpp", [P, 2 * C], F16, kind="ExternalInput").ap()
    cc_d = nc.dram_tensor("cc", [P, T], F16, kind="ExternalInput").ap()
    ss_d = nc.dram_tensor("ss", [P, T], F16, kind="ExternalInput").ap()
    ebc_d = nc.dram_tensor("ebc", [P, 2 * P], F32R, kind="ExternalInput").ap()
    y_d = nc.dram_tensor("y", [T, C], F32, kind="ExternalOutput").ap()

    SHUF_MASK = [i ^ 16 for i in range(32)]
    NBW = KC * TQ  # 4096: x columns per token block

    with tile.TileContext(nc) as tc:
        with (
            tc.tile_pool(name="consts", bufs=1) as consts,
            tc.tile_pool(name="bigs", bufs=1) as bigs,
            tc.tile_pool(name="tmps", bufs=2) as tmps,
            tc.tile_pool(name="expool", bufs=4) as expool,
            tc.tile_pool(name="psS", bufs=2, space="PSUM") as psS,
            tc.tile_pool(name="psW", bufs=2, space="PSUM") as psW,
            tc.tile_pool(name="psO", bufs=2, space="PSUM") as psO,
        ):
            # ---- resident tiles ----
            x_big = bigs.tile([P, KC * T], F16, tag="xbig", name="xbig")
            wqkk_t = bigs.tile([P, KC * 2 * P], F16, tag="wqkk", name="wqkk")
            wqkq_t = bigs.tile([P, KC * 2 * P], F16, tag="wqkq", name="wqkq")
            wv_big = bigs.tile([P, KC * GC], F16, tag="wvbig", name="wvbig")
            wp_big = bigs.tile([P, 2 * C], F16, tag="wpbig", name="wpbig")
            cc_t = consts.tile([P, T], F16, tag="cc")
            ss_t = consts.tile([P, T], F16, tag="ss")
            ebc_t = consts.tile([P, 2 * P], F32R, tag="ebc")

            # ---- warmup: ramp the PE p-state and preload the ACT exp
            # table during the DMA wait. exp(0*x)=1 makes the ones tile.
            warm = consts.tile([P, TQ], F16, tag="warm")
            nc.vector.memset(warm, 0.0)
            wps = psW.tile([P, TQ], F32, tag="aux", name="warmps")
            for i in range(4):
                nc.tensor.matmul(
                    wps, lhsT=warm[:, 0:P], rhs=warm, start=(i == 0), stop=(i == 3)
                )
            ones_f = consts.tile([P, TQ], F32, tag="ones_f")
            nc.scalar.activation(
                out=ones_f,
                in_=wps,
                func=mybir.ActivationFunctionType.Exp,
                scale=0.0,
            )
            ones4 = ones_f[:, 0 : 2 * HPC].rearrange("p (h c) -> p h c", c=2)

            # ---- DMAs in priority order (deps of early compute first) ----
            nc.sync.dma_start(out=wqkk_t, in_=wqkk_d)
            nc.sync.dma_start(out=x_big[:, 0:NBW], in_=xb_d[:, 0:NBW])  # x n0
            nc.sync.dma_start(out=cc_t[:, 0:TQ], in_=cc_d[:, 0:TQ])
            nc.sync.dma_start(out=ss_t[:, 0:TQ], in_=ss_d[:, 0:TQ])
            nc.sync.dma_start(out=wqkq_t, in_=wqkq_d)
            nc.sync.dma_start(out=wv_big, in_=wvp_d)
            for nb in range(1, NQB):  # x n1..n3, landing just in time
                nc.sync.dma_start(
                    out=x_big[:, nb * NBW : (nb + 1) * NBW],
                    in_=xb_d[:, nb * NBW : (nb + 1) * NBW],
                )
            nc.sync.dma_start(out=cc_t[:, TQ:T], in_=cc_d[:, TQ:T])
            nc.sync.dma_start(out=ss_t[:, TQ:T], in_=ss_d[:, TQ:T])
            nc.sync.dma_start(out=wp_big, in_=wpp_d)
            nc.sync.dma_start(out=ebc_t, in_=ebc_d)

            qk_sb = [
                bigs.tile([P, T], F16, tag=f"qk{mb}", name=f"qk{mb}")
                for mb in range(4)
            ]
            va_list = [
                bigs.tile([P, HPC * VW], F16, tag=f"va{tb}", name=f"va{tb}")
                for tb in range(NKB)
            ]

            def xsl(kc, n):
                """x slice [128, TQ] for contraction chunk kc, token block n"""
                base = n * NBW + kc * TQ
                return x_big[:, base : base + TQ]

            # ---- work-unit emitters ----
            def proj_qk_tile(mb, n):
                """One qk projection tile + RoPE (atomic filler unit)."""
                ns = slice(n * TQ, (n + 1) * TQ)
                wsrc = wqkq_t if mb < 2 else wqkk_t
                mo = (mb % 2) * P
                ps = psW.tile([P, TQ], F32, tag="aux", name=f"ps{mb}_{n}")
                for kc in range(KC):
                    nc.tensor.matmul(
                        ps,
                        lhsT=wsrc[:, kc * 2 * P + mo : kc * 2 * P + mo + P],
                        rhs=xsl(kc, n),
                        start=(kc == 0),
                        stop=(kc == KC - 1),
                    )
                sb = qk_sb[mb]
                nc.vector.tensor_copy(sb[:, ns], ps)
                shuf = tmps.tile([P, TQ], F16, tag="shuf")
                nc.vector.stream_shuffle(shuf, sb[:, ns], SHUF_MASK)
                nc.vector.tensor_mul(sb[:, ns], sb[:, ns], cc_t[:, ns])
                tmp = tmps.tile([P, TQ], F16, tag="ropetmp")
                nc.vector.tensor_mul(tmp, shuf, ss_t[:, ns])
                nc.vector.tensor_add(sb[:, ns], sb[:, ns], tmp)

            def proj_v_tb(tb):
                vp = psW.tile([P, TQ], F32, tag="aux", name=f"vp{tb}")
                vps = vp[:, 0:GC]
                xoff = (tb // 4) * NBW + (tb % 4) * P
                for kc in range(KC):
                    nc.tensor.matmul(
                        vps,
                        lhsT=x_big[:, xoff + kc * TQ : xoff + kc * TQ + P],
                        rhs=wv_big[:, kc * GC : (kc + 1) * GC],
                        start=(kc == 0),
                        stop=(kc == KC - 1),
                    )
                va = va_list[tb]
                va4 = va.rearrange("p (h c) -> p h c", c=VW)
                nc.vector.tensor_copy(va4[:, :, HD : HD + 2], ones4)
                nc.vector.tensor_copy(
                    va4[:, :, 0:HD], vps.rearrange("p (h c) -> p h c", c=HD)
                )

            # ---- deadline-scheduled filler pump ----
            fillers = []  # heap of (deadline, seq, cost, fn)
            fseq = [0]
            credit = [0.0]
            ACT_NS = 1150.0
            BASE_NS = 820.0

            def add_filler(deadline, cost, fn):
                heapq.heappush(fillers, (deadline, fseq[0], cost, fn))
                fseq[0] += 1

            def pump(j):
                credit[0] = min(credit[0] + (ACT_NS - BASE_NS), 2400.0)
                while fillers and fillers[0][0] <= j:
                    _, _, c, fn = heapq.heappop(fillers)
                    fn()
                    credit[0] -= c
                credit[0] = max(credit[0], -1500.0)
                while fillers and credit[0] >= fillers[0][2]:
                    _, _, c, fn = heapq.heappop(fillers)
                    fn()
                    credit[0] -= c

            # ---- pipelined attention stages over global ticks ----
            # tick g = qb*32 + p*16 + kb
            st2_live = {}
            ex_live = {}
            oau_live = {}
            qdat = {}  # qb -> (oevp, den4)
            odat = {}  # qb -> [o_sb0, o_sb1]

            def scores_stage(g):
                qb, r = divmod(g, 32)
                p, kb = divmod(r, 16)
                qs = slice(qb * TQ, (qb + 1) * TQ)
                ks = slice(kb * P, (kb + 1) * P)
                qt = qk_sb[p]
                kt = qk_sb[2 + p]
                st2 = psS.tile([P, 2 * TQ], F32, tag="st2", name=f"st2_{g}")
                for i in range(2):
                    nc.tensor.matmul(
                        st2[:, i * TQ : (i + 1) * TQ],
                        lhsT=kt[i * HD : (i + 1) * HD, ks],
                        rhs=qt[i * HD : (i + 1) * HD, qs],
                        start=True,
                        stop=True,
                    )
                st2_live[g] = st2

            def exp_stage(g):
                st2 = st2_live.pop(g)
                ex = expool.tile([P, 2 * TQ], F16, tag="ex", name=f"ex_{g}")
                nc.scalar.activation(
                    out=ex,
                    in_=st2,
                    func=mybir.ActivationFunctionType.Exp,
                    scale=1.0 / np.sqrt(HD),
                )
                ex_live[g] = ex

            def av_stage(g):
                qb, r = divmod(g, 32)
                p, kb = divmod(r, 16)
                if p == 0 and kb == 0:
                    oevp = [
                        tmps.tile(
                            [P, TQ], F32, tag=f"oevp{pp}",
                            name=f"oevp{pp}_{qb}", bufs=2,
                        )
                        for pp in range(2)
                    ]
                    den4 = tmps.tile(
                        [P, TQ], F32, tag="den4", name=f"den4_{qb}", bufs=2
                    )
                    nc.vector.memset(den4, 1.0)
                    qdat[qb] = (oevp, den4)
                if kb == 0:
                    oau_live[(qb, p)] = [
                        psO.tile([VW, TQ], F32, tag="oau", name=f"oau{i}_{qb}{p}")
                        for i in range(2)
                    ]
                oau = oau_live[(qb, p)]
                ex = ex_live.pop(g)
                for i in range(2):
                    h = 2 * p + i
                    nc.tensor.matmul(
                        oau[i],
                        lhsT=va_list[kb][:, h * VW : h * VW + VW],
                        rhs=ex[:, i * TQ : (i + 1) * TQ],
                        start=(kb == 0),
                        stop=(kb == NKB - 1),
                    )
                if kb == NKB - 1:
                    oevp, den4 = qdat[qb]
                    for i in range(2):
                        nc.vector.tensor_copy(
                            oevp[p][i * HD : (i + 1) * HD, :], oau[i][0:HD, :]
                        )
                        r0 = 32 * (2 * p + i)
                        nc.vector.tensor_copy(
                            den4[r0 : r0 + 1, :], oau[i][HD : HD + 1, :]
                        )
                    del oau_live[(qb, p)]
                    if p == 0:
                        add_filler(qb * 32 + 22, 900.0, lambda q=qb: fin_pair(q, 0))
                    else:
                        add_filler(qb * 32 + 36, 900.0, lambda q=qb: fin_pair(q, 1))
                        for tch in range(TQ // P):
                            add_filler(
                                qb * 32 + 38 + 4 * tch,
                                1300.0,
                                lambda q=qb, t=tch: yproj_tch(q, t),
                            )

            # ---- finalize: full-width reciprocal + 0/1 broadcast matmul ----
            def fin_pair(qb, p):
                oevp, den4 = qdat[qb]
                o_sb = tmps.tile(
                    [P, TQ], F16, tag=f"osb{p}", name=f"osb{p}_{qb}", bufs=2
                )
                odat.setdefault(qb, [None, None])[p] = o_sb
                # den4 rows off this pair's heads hold 1.0 (memset) so the
                # full-width reciprocal is safe; ebc's zeros mask them out.
                rden = tmps.tile([P, TQ], F32, tag="rden")
                nc.vector.reciprocal(rden, den4)
                rden_r = tmps.tile([P, TQ], F32R, tag="rdenr")
                with nc.allow_low_precision(reason="f32r round of 1/den"):
                    nc.vector.tensor_copy(rden_r, rden)
                bc = psW.tile([P, TQ], F32, tag="aux", name=f"bc{qb}{p}")
                nc.tensor.matmul(
                    bc,
                    lhsT=ebc_t[:, p * P : (p + 1) * P],
                    rhs=rden_r,
                    start=True,
                    stop=True,
                )
                nc.vector.tensor_mul(o_sb, oevp[p], bc)
                if p == 1:
                    del qdat[qb]

            def yproj_tch(qb, tch):
                o_sb = odat[qb]
                yp = psS.tile([P, 2 * TQ], F32, tag="st2", name=f"yp{qb}{tch}")
                for cch in range(2):  # matmul out must stay within one PSUM bank
                    for kb in range(2):
                        nc.tensor.matmul(
                            yp[:, cch * TQ : (cch + 1) * TQ],
                            lhsT=o_sb[kb][:, tch * P : (tch + 1) * P],
                            rhs=wp_big[:, kb * C + cch * TQ : kb * C + (cch + 1) * TQ],
                            start=(kb == 0),
                            stop=(kb == 1),
                        )
                ysb = tmps.tile([P, 2 * TQ], F32, tag="ysb")
                if qb == NQB - 1 and tch % 2 == 1:
                    # tail: ACT engine is idle, split the staging copies
                    nc.scalar.copy(ysb, yp)
                else:
                    nc.vector.tensor_copy(ysb, yp)
                r0 = qb * TQ + tch * P
                nc.sync.dma_start(out=y_d[r0 : r0 + P, :], in_=ysb)
                if tch == TQ // P - 1:
                    del odat[qb]

            # ---- prologue: k(pair0, n0) and q(pair0, n0) directly ----
            proj_qk_tile(2, 0)
            proj_qk_tile(0, 0)

            # ---- seed filler units with deadlines (j-space) ----
            for tb in range(NKB):
                add_filler(max(tb, 1), 1250.0, lambda t=tb: proj_v_tb(t))
            for nb in range(1, NQB):
                add_filler(4 * nb - 1, 2100.0, lambda n=nb: proj_qk_tile(2, n))
            add_filler(13, 2100.0, lambda: proj_qk_tile(3, 0))
            for nb in range(1, NQB):
                add_filler(16 + 4 * nb - 1, 2100.0, lambda n=nb: proj_qk_tile(3, n))
            add_filler(14, 2100.0, lambda: proj_qk_tile(1, 0))
            for nb in range(1, NQB):
                add_filler(32 * nb - 3, 2100.0, lambda n=nb: proj_qk_tile(0, n))
                add_filler(32 * nb + 13, 2100.0, lambda n=nb: proj_qk_tile(1, n))

            # ---- main pipelined loop ----
            for j in range(NT + 2):
                pump(j)
                if j < NT:
                    scores_stage(j)
                if 1 <= j <= NT:
                    exp_stage(j - 1)
                if 2 <= j:
                    av_stage(j - 2)
            while fillers:
                _, _, _, fn = heapq.heappop(fillers)
                fn()

    nc.compile()
    return nc


def _get_program():
    global _PROGRAM
    if _PROGRAM is None:
        _PROGRAM = _build_program()
    return _PROGRAM


def _eo(w):
    """[64, C] head rows -> [even0:16; odd0:16; even16:32; odd16:32] so the
    RoPE partner is at p^16 within a 32-partition block (stream_shuffle)."""
    e = w[0::2]
    o = w[1::2]
    return np.concatenate([e[0:16], o[0:16], e[16:32], o[16:32]], axis=0)


def _pack_rows(a):
    """[n*128, F] (chunk-major rows) -> [128, n*F] partition-major so the
    DMA is one contiguous 2D copy."""
    nch = a.shape[0] // P
    kcf = a.reshape(nch, P, -1)  # [chunk, p, F]
    return np.ascontiguousarray(np.transpose(kcf, (1, 0, 2)).reshape(P, -1))


def _host_prep(x, cos, sin, w_qkv, w_proj):
    """Build the 8 per-core input maps (everything pre-packed to the SBUF
    partition-major layout)."""
    f16 = np.float16

    # x: [C, T] kc-major rows -> [128, nb-major (kc, t-block)] columns
    xbs = []
    for b in range(B):
        xT = x[b].T.reshape(KC, P, NQB, TQ)  # [kc, p, nb, t]
        xb = np.transpose(xT, (1, 2, 0, 3)).reshape(P, -1)  # [p, nb*kc*TQ]
        xbs.append(np.ascontiguousarray(xb).astype(f16))

    cosT = np.ascontiguousarray(cos.T)  # [32, T]
    sinT = np.ascontiguousarray(sin.T)
    cc64 = np.concatenate([cosT[0:16], cosT[0:16], cosT[16:32], cosT[16:32]])
    ss64 = np.concatenate([-sinT[0:16], sinT[0:16], -sinT[16:32], sinT[16:32]])
    cc = np.tile(cc64, (2, 1)).astype(f16)  # [128, T]
    ss = np.tile(ss64, (2, 1)).astype(f16)

    # ebc[k, p*128+m] = 1 where k == 32*(2p + m//64): broadcasts den row
    ebc = np.zeros((P, 2 * P), dtype=np.float32)
    for p in range(2):
        for i in range(2):
            ebc[32 * (2 * p + i), p * P + i * HD : p * P + (i + 1) * HD] = 1.0

    wq = w_qkv[0:C]
    wk = w_qkv[C : 2 * C]
    wv = w_qkv[2 * C : 3 * C]

    in_maps = []
    for core in range(N_CORES):
        b = core // 4
        h0 = 4 * (core % 4)
        heads = [h0, h0 + 1, h0 + 2, h0 + 3]

        def pair_block(w, pair):
            ha, hb = heads[2 * pair], heads[2 * pair + 1]
            return np.concatenate(
                [_eo(w[ha * HD : ha * HD + HD]), _eo(w[hb * HD : hb * HD + HD])],
                axis=0,
            )  # [128, C]

        # per-kc [128, 2P] blocks: cols = [pair0 | pair1] m-offsets
        wq2 = np.concatenate([pair_block(wq, 0), pair_block(wq, 1)], axis=0)  # [256,C]
        wk2 = np.concatenate([pair_block(wk, 0), pair_block(wk, 1)], axis=0)
        # rows are m (proj channel), cols C (contraction): want [128p, kc*2P]
        # with lhsT[p=contraction chunk rows, m]: lhsT col-block kc holds
        # w[m, kc*128+p] at [p, kc*256 + m]
        wqkq = np.ascontiguousarray(
            np.transpose(wq2.T.reshape(KC, P, 2 * P), (1, 0, 2)).reshape(P, -1)
        ).astype(f16)
        wqkk = np.ascontiguousarray(
            np.transpose(wk2.T.reshape(KC, P, 2 * P), (1, 0, 2)).reshape(P, -1)
        ).astype(f16)
        wvp = _pack_rows(
            np.ascontiguousarray(wv[h0 * HD : h0 * HD + GC].T)  # [C, 256]
        ).astype(f16)
        wpp = _pack_rows(
            np.ascontiguousarray(w_proj[:, h0 * HD : h0 * HD + GC].T)  # [256, C]
        ).astype(f16)
        in_maps.append(
            {
                "xb": xbs[b],
                "wqkk": wqkk,
                "wqkq": wqkq,
                "wvp": wvp,
                "wpp": wpp,
                "cc": cc,
                "ss": ss,
                "ebc": ebc,
            }
        )
    return in_maps


def kernel(x, cos, sin, mask, w_qkv, w_proj, _trace=False, _tmpdir=None):
    x = np.asarray(x, dtype=np.float32)
    cos = np.asarray(cos, dtype=np.float32)
    sin = np.asarray(sin, dtype=np.float32)
    w_qkv = np.asarray(w_qkv, dtype=np.float32)
    w_proj = np.asarray(w_proj, dtype=np.float32)
    # mask is all-ones in this problem spec: no-op in the math.

    nc = _get_program()
    in_maps = _host_prep(x, cos, sin, w_qkv, w_proj)
    res = run_bass_kernel_spmd(
        nc, in_maps, list(range(N_CORES)), trace=_trace, tmpdir=_tmpdir
    )
    out = np.empty((B, T, C), dtype=np.float32)
    for b in range(B):
        acc = res.results[4 * b]["y"].astype(np.float32).copy()
        for g in range(1, 4):
            acc += res.results[4 * b + g]["y"]
        out[b] = acc
    kernel._last_exec_time_ns = res.exec_time_ns
    return out


# revision 21
# speedup vs baseline: 1.1295x; 1.1295x over previous
"""Multi-head attention (B=2, T=2048, C=1024, H=16, hd=64, RoPE, full mask)
on 8 TRN2 NeuronCores.

Sharding: tensor-parallel over (batch, head-group). Core c handles batch
c//4 and heads [4*(c%4) .. 4*(c%4)+3]. Each core computes the QKV
projection for its 4 heads, full attention over T=2048, and a partial
output projection y = o_heads @ w_proj[:, cols].T. The host sums the 4
partial y's per batch (the tensor-parallel unshard reduction).

On-chip layout is "transposed everything" so attention needs no on-chip
transposes of the big tensors:
  - qT/kT stored [head_dim, T] (projection computed as w @ x.T)
  - scores computed directly transposed: sT[tk, tq] = k[tk] . q[tq]
  - softmax denominator via an appended ones-column on V (M=66 matmul)
  - o.T scaled by 1/den via full-width DVE reciprocal + one 0/1
    broadcast matmul (ebc)
RoPE uses full-width elementwise ops after a host-side row permutation
of w_q/w_k plus a DVE stream_shuffle; stream_shuffle permutes within
32-partition blocks, so rows are laid [e0:16, o0:16, e16:32, o16:32]
putting the rotation partner at p^16.

All inputs are pre-packed on the HOST into the exact SBUF partition-major
layout, so every DMA is a contiguous 2D copy with 4-8KB lines (3D
scatter patterns are descriptor-bound and 5-10x slower).

Schedule: the scalar-engine exp over the T^2 scores (~142us/core) is the
binding resource. The program is ONE globally software-pipelined tick
stream over all 128 (qb, pair, kb) ticks with the scores matmul running
one tick AHEAD of the exp and AV one tick behind:
    tick j:  [pump fillers] scores(j) | exp(j-1) | AV(j-2)
so filler work pumped between ticks can never stall the exp stream.
Everything outside the prologue (k(n0), q(qb0)) is a filler unit with a
deadline: remaining qkv projections, RoPE, per-pair finalize chains, and
output-projection chunks. Priority-ordered DMAs + PE/ACT warmup shrink
the head; the per-pair finalize and ACT-assisted output copies shrink
the tail.

Precision: f16 operands with fp32 PSUM accumulation everywhere.
"""

import heapq

import ml_dtypes  # noqa: F401
import numpy as np

import concourse.bacc as bacc
import concourse.mybir as mybir
import concourse.tile as tile
from concourse.bass_utils import run_bass_kernel_spmd

# Problem constants (hardcoded per contract)
B, T, C = 2, 2048, 1024
N_HEAD = 16
HD = 64
N_CORES = 8
HPC = 4  # heads per core
GC = HPC * HD  # head channels per core = 256

P = 128
KC = C // P  # 8 contraction chunks for the projections
NQB = 4  # query blocks
TQ = T // NQB  # 512
NKB = T // P  # 16 key blocks
VW = HD + 2  # 66: v + ones col + pad col
NT = 4 * 32  # total ticks

F32 = mybir.dt.float32
F32R = mybir.dt.float32r
F16 = mybir.dt.float16

_PROGRAM = None


def _build_program():
    nc = bacc.Bacc(
        "TRN2", target_bir_lowering=False, debug=False, num_devices=N_CORES
    )

    # all host-prepacked to SBUF layout: plain contiguous 2D DMAs
    xb_d = nc.dram_tensor("xb", [P, KC * T], F16, kind="ExternalInput").ap()
    wqkk_d = nc.dram_tensor("wqkk", [P, KC * 2 * P], F16, kind="ExternalInput").ap()
    wqkq_d = nc.dram_tensor("wqkq", [P, KC * 2 * P], F16, kind="ExternalInput").ap()
    wvp_d = nc.dram_tensor("wvp", [P, KC * GC], F16, kind="ExternalInput").ap()
    wpp_d = nc.dram_tensor("wpp", [P, 2 * C], F16, kind="ExternalInput").ap()
    cc_d = nc.dram_tensor("cc", [P, T], F16, kind="ExternalInput").ap()
    ss_d = nc.dram_tensor("ss", [P, T], F16, kind="ExternalInput").ap()
    ebc_d = nc.dram_tensor("ebc", [P, 2 * P], F32R, kind="ExternalInput").ap()
    y_d = nc.dram_tensor("y", [T, C], F32, kind="ExternalOutput").ap()

    SHUF_MASK = [i ^ 16 for i in range(32)]
    NBW = KC * TQ  # 4096: x columns per token block

    with tile.TileContext(nc) as tc:
        with (
            tc.tile_pool(name="consts", bufs=1) as consts,
            tc.tile_pool(name="bigs", bufs=1) as bigs,
            tc.tile_pool(name="tmps", bufs=2) as tmps,
            tc.tile_pool(name="expool", bufs=4) as expool,
            tc.tile_pool(name="psS", bufs=2, space="PSUM") as psS,
            tc.tile_pool(name="psW", bufs=2, space="PSUM") as psW,
            tc.tile_pool(name="psO", bufs=2, space="PSUM") as psO,
        ):
            # ---- resident tiles ----
            x_big = bigs.tile([P, KC * T], F16, tag="xbig", name="xbig")
            wqkk_t = bigs.tile([P, KC * 2 * P], F16, tag="wqkk", name="wqkk")
            wqkq_t = bigs.tile([P, KC * 2 * P], F16, tag="wqkq", name="wqkq")
            wv_big = bigs.tile([P, KC * GC], F16, tag="wvbig", name="wvbig")
            wp_big = bigs.tile([P, 2 * C], F16, tag="wpbig", name="wpbig")
            cc_t = consts.tile([P, T], F16, tag="cc")
            ss_t = consts.tile([P, T], F16, tag="ss")
            ebc_t = consts.tile([P, 2 * P], F32R, tag="ebc")

            # ---- warmup: ramp the PE p-state and preload the ACT exp
            # table during the DMA wait. exp(0*x)=1 makes the ones tile.
            warm = consts.tile([P, TQ], F16, tag="warm")
            nc.vector.memset(warm, 0.0)
            wps = psW.tile([P, TQ], F32, tag="aux", name="warmps")
            NWARM = 12  # long enough to keep the PE p-state hot until x lands
            for i in range(NWARM):
                nc.tensor.matmul(
                    wps,
                    lhsT=warm[:, 0:P],
                    rhs=warm,
                    start=(i == 0),
                    stop=(i == NWARM - 1),
                )
            ones_f = consts.tile([P, TQ], F32, tag="ones_f")
            nc.scalar.activation(
                out=ones_f,
                in_=wps,
                func=mybir.ActivationFunctionType.Exp,
                scale=0.0,
            )
            ones4 = ones_f[:, 0 : 2 * HPC].rearrange("p (h c) -> p h c", c=2)

            # ---- DMAs in priority order (deps of early compute first) ----
            # x n0 in halves: k-proj's kc-accumulation starts on half 0.
            nc.sync.dma_start(out=x_big[:, 0 : NBW // 2], in_=xb_d[:, 0 : NBW // 2])
            nc.sync.dma_start(out=wqkk_t, in_=wqkk_d)
            nc.sync.dma_start(
                out=x_big[:, NBW // 2 : NBW], in_=xb_d[:, NBW // 2 : NBW]
            )
            nc.sync.dma_start(out=cc_t[:, 0:TQ], in_=cc_d[:, 0:TQ])
            nc.sync.dma_start(out=ss_t[:, 0:TQ], in_=ss_d[:, 0:TQ])
            nc.sync.dma_start(out=wqkq_t, in_=wqkq_d)
            nc.sync.dma_start(out=wv_big, in_=wvp_d)
            for nb in range(1, NQB):  # x n1..n3, landing just in time
                nc.sync.dma_start(
                    out=x_big[:, nb * NBW : (nb + 1) * NBW],
                    in_=xb_d[:, nb * NBW : (nb + 1) * NBW],
                )
            nc.sync.dma_start(out=cc_t[:, TQ:T], in_=cc_d[:, TQ:T])
            nc.sync.dma_start(out=ss_t[:, TQ:T], in_=ss_d[:, TQ:T])
            nc.sync.dma_start(out=wp_big, in_=wpp_d)
            nc.sync.dma_start(out=ebc_t, in_=ebc_d)

            qk_sb = [
                bigs.tile([P, T], F16, tag=f"qk{mb}", name=f"qk{mb}")
                for mb in range(4)
            ]
            va_list = [
                bigs.tile([P, HPC * VW], F16, tag=f"va{tb}", name=f"va{tb}")
                for tb in range(NKB)
            ]

            def xsl(kc, n):
                """x slice [128, TQ] for contraction chunk kc, token block n"""
                base = n * NBW + kc * TQ
                return x_big[:, base : base + TQ]

            # ---- work-unit emitters ----
            def proj_qk_tile(mb, n):
                """One qk projection tile + RoPE (atomic filler unit)."""
                ns = slice(n * TQ, (n + 1) * TQ)
                wsrc = wqkq_t if mb < 2 else wqkk_t
                mo = (mb % 2) * P
                ps = psW.tile([P, TQ], F32, tag="aux", name=f"ps{mb}_{n}")
                for kc in range(KC):
                    nc.tensor.matmul(
                        ps,
                        lhsT=wsrc[:, kc * 2 * P + mo : kc * 2 * P + mo + P],
                        rhs=xsl(kc, n),
                        start=(kc == 0),
                        stop=(kc == KC - 1),
                    )
                sb = qk_sb[mb]
                nc.vector.tensor_copy(sb[:, ns], ps)
                shuf = tmps.tile([P, TQ], F16, tag="shuf")
                nc.vector.stream_shuffle(shuf, sb[:, ns], SHUF_MASK)
                nc.vector.tensor_mul(sb[:, ns], sb[:, ns], cc_t[:, ns])
                tmp = tmps.tile([P, TQ], F16, tag="ropetmp")
                nc.vector.tensor_mul(tmp, shuf, ss_t[:, ns])
                nc.vector.tensor_add(sb[:, ns], sb[:, ns], tmp)

            def proj_v_tb(tb):
                vp = psW.tile([P, TQ], F32, tag="aux", name=f"vp{tb}")
                vps = vp[:, 0:GC]
                xoff = (tb // 4) * NBW + (tb % 4) * P
                for kc in range(KC):
                    nc.tensor.matmul(
                        vps,
                        lhsT=x_big[:, xoff + kc * TQ : xoff + kc * TQ + P],
                        rhs=wv_big[:, kc * GC : (kc + 1) * GC],
                        start=(kc == 0),
                        stop=(kc == KC - 1),
                    )
                va = va_list[tb]
                va4 = va.rearrange("p (h c) -> p h c", c=VW)
                nc.vector.tensor_copy(va4[:, :, HD : HD + 2], ones4)
                nc.vector.tensor_copy(
                    va4[:, :, 0:HD], vps.rearrange("p (h c) -> p h c", c=HD)
                )

            # ---- deadline-scheduled filler pump ----
            fillers = []  # heap of (deadline, seq, cost, fn)
            fseq = [0]
            credit = [0.0]
            ACT_NS = 1150.0
            BASE_NS = 820.0

            def add_filler(deadline, cost, fn):
                heapq.heappush(fillers, (deadline, fseq[0], cost, fn))
                fseq[0] += 1

            def pump(j):
                credit[0] = min(credit[0] + (ACT_NS - BASE_NS), 2400.0)
                while fillers and fillers[0][0] <= j:
                    _, _, c, fn = heapq.heappop(fillers)
                    fn()
                    credit[0] -= c
                credit[0] = max(credit[0], -1500.0)
                while fillers and credit[0] >= fillers[0][2]:
                    _, _, c, fn = heapq.heappop(fillers)
                    fn()
                    credit[0] -= c

            # ---- pipelined attention stages over global ticks ----
            # tick g = qb*32 + p*16 + kb
            st2_live = {}
            ex_live = {}
            oau_live = {}
            qdat = {}  # qb -> (oevp, den4)
            odat = {}  # qb -> [o_sb0, o_sb1]

            def scores_stage(g):
                qb, r = divmod(g, 32)
                p, kb = divmod(r, 16)
                qs = slice(qb * TQ, (qb + 1) * TQ)
                ks = slice(kb * P, (kb + 1) * P)
                qt = qk_sb[p]
                kt = qk_sb[2 + p]
                st2 = psS.tile([P, 2 * TQ], F32, tag="st2", name=f"st2_{g}")
                for i in range(2):
                    nc.tensor.matmul(
                        st2[:, i * TQ : (i + 1) * TQ],
                        lhsT=kt[i * HD : (i + 1) * HD, ks],
                        rhs=qt[i * HD : (i + 1) * HD, qs],
                        start=True,
                        stop=True,
                    )
                st2_live[g] = st2

            def exp_stage(g):
                st2 = st2_live.pop(g)
                ex = expool.tile([P, 2 * TQ], F16, tag="ex", name=f"ex_{g}")
                nc.scalar.activation(
                    out=ex,
                    in_=st2,
                    func=mybir.ActivationFunctionType.Exp,
                    scale=1.0 / np.sqrt(HD),
                )
                ex_live[g] = ex

            def av_stage(g):
                qb, r = divmod(g, 32)
                p, kb = divmod(r, 16)
                if p == 0 and kb == 0:
                    oevp = [
                        tmps.tile(
                            [P, TQ], F32, tag=f"oevp{pp}",
                            name=f"oevp{pp}_{qb}", bufs=2,
                        )
                        for pp in range(2)
                    ]
                    den4 = tmps.tile(
                        [P, TQ], F32, tag="den4", name=f"den4_{qb}", bufs=2
                    )
                    nc.vector.memset(den4, 1.0)
                    qdat[qb] = (oevp, den4)
                if kb == 0:
                    oau_live[(qb, p)] = [
                        psO.tile([VW, TQ], F32, tag="oau", name=f"oau{i}_{qb}{p}")
                        for i in range(2)
                    ]
                oau = oau_live[(qb, p)]
                ex = ex_live.pop(g)
                for i in range(2):
                    h = 2 * p + i
                    nc.tensor.matmul(
                        oau[i],
                        lhsT=va_list[kb][:, h * VW : h * VW + VW],
                        rhs=ex[:, i * TQ : (i + 1) * TQ],
                        start=(kb == 0),
                        stop=(kb == NKB - 1),
                    )
                if kb == NKB - 1:
                    oevp, den4 = qdat[qb]
                    tail = qb == NQB - 1 and p == 1
                    for i in range(2):
                        if tail:  # ACT engine is idle after the last exp
                            nc.scalar.copy(
                                oevp[p][i * HD : (i + 1) * HD, :], oau[i][0:HD, :]
                            )
                        else:
                            nc.vector.tensor_copy(
                                oevp[p][i * HD : (i + 1) * HD, :], oau[i][0:HD, :]
                            )
                        r0 = 32 * (2 * p + i)
                        nc.vector.tensor_copy(
                            den4[r0 : r0 + 1, :], oau[i][HD : HD + 1, :]
                        )
                    del oau_live[(qb, p)]
                    if p == 0:
                        add_filler(qb * 32 + 22, 900.0, lambda q=qb: fin_pair(q, 0))
                    else:
                        add_filler(qb * 32 + 36, 900.0, lambda q=qb: fin_pair(q, 1))
                        for tch in range(TQ // P):
                            add_filler(
                                qb * 32 + 38 + 4 * tch,
                                1300.0,
                                lambda q=qb, t=tch: yproj_tch(q, t),
                            )

            # ---- finalize: full-width reciprocal + 0/1 broadcast matmul ----
            def fin_pair(qb, p):
                oevp, den4 = qdat[qb]
                o_sb = tmps.tile(
                    [P, TQ], F16, tag=f"osb{p}", name=f"osb{p}_{qb}", bufs=2
                )
                odat.setdefault(qb, [None, None])[p] = o_sb
                # den4 rows off this pair's heads hold 1.0 (memset) so the
                # full-width reciprocal is safe; ebc's zeros mask them out.
                rden = tmps.tile([P, TQ], F32, tag="rden")
                nc.vector.reciprocal_approx_fast(rden, den4)
                rden_r = tmps.tile([P, TQ], F32R, tag="rdenr")
                with nc.allow_low_precision(reason="f32r round of 1/den"):
                    nc.vector.tensor_copy(rden_r, rden)
                bc = psW.tile([P, TQ], F32, tag="aux", name=f"bc{qb}{p}")
                nc.tensor.matmul(
                    bc,
                    lhsT=ebc_t[:, p * P : (p + 1) * P],
                    rhs=rden_r,
                    start=True,
                    stop=True,
                )
                nc.vector.tensor_mul(o_sb, oevp[p], bc)
                if p == 1:
                    del qdat[qb]

            def yproj_tch(qb, tch):
                o_sb = odat[qb]
                yp = psS.tile([P, 2 * TQ], F32, tag="st2", name=f"yp{qb}{tch}")
                for cch in range(2):  # matmul out must stay within one PSUM bank
                    for kb in range(2):
                        nc.tensor.matmul(
                            yp[:, cch * TQ : (cch + 1) * TQ],
                            lhsT=o_sb[kb][:, tch * P : (tch + 1) * P],
                            rhs=wp_big[:, kb * C + cch * TQ : kb * C + (cch + 1) * TQ],
                            start=(kb == 0),
                            stop=(kb == 1),
                        )
                ysb = tmps.tile([P, 2 * TQ], F32, tag="ysb")
                if qb == NQB - 1 and tch % 2 == 1:
                    # tail: ACT engine is idle, split the staging copies
                    nc.scalar.copy(ysb, yp)
                else:
                    nc.vector.tensor_copy(ysb, yp)
                r0 = qb * TQ + tch * P
                nc.sync.dma_start(out=y_d[r0 : r0 + P, :], in_=ysb)
                if tch == TQ // P - 1:
                    del odat[qb]

            # ---- prologue: k(pair0, n0) and q(pair0, n0) directly ----
            proj_qk_tile(2, 0)
            proj_qk_tile(0, 0)

            # ---- seed filler units with deadlines (j-space) ----
            for tb in range(NKB):
                add_filler(max(tb, 1), 1250.0, lambda t=tb: proj_v_tb(t))
            for nb in range(1, NQB):
                add_filler(4 * nb - 1, 2100.0, lambda n=nb: proj_qk_tile(2, n))
            add_filler(13, 2100.0, lambda: proj_qk_tile(3, 0))
            for nb in range(1, NQB):
                add_filler(16 + 4 * nb - 1, 2100.0, lambda n=nb: proj_qk_tile(3, n))
            add_filler(14, 2100.0, lambda: proj_qk_tile(1, 0))
            for nb in range(1, NQB):
                add_filler(32 * nb - 3, 2100.0, lambda n=nb: proj_qk_tile(0, n))
                add_filler(32 * nb + 13, 2100.0, lambda n=nb: proj_qk_tile(1, n))

            # ---- main pipelined loop ----
            for j in range(NT + 2):
                pump(j)
                if j < NT:
                    scores_stage(j)
                if 1 <= j <= NT:
                    exp_stage(j - 1)
                if 2 <= j:
                    av_stage(j - 2)
            while fillers:
                _, _, _, fn = heapq.heappop(fillers)
                fn()

    nc.compile()
    return nc


def _get_program():
    global _PROGRAM
    if _PROGRAM is None:
        _PROGRAM = _build_program()
    return _PROGRAM


def _eo(w):
    """[64, C] head rows -> [even0:16; odd0:16; even16:32; odd16:32] so the
    RoPE partner is at p^16 within a 32-partition block (stream_shuffle)."""
    e = w[0::2]
    o = w[1::2]
    return np.concatenate([e[0:16], o[0:16], e[16:32], o[16:32]], axis=0)


def _pack_rows(a):
    """[n*128, F] (chunk-major rows) -> [128, n*F] partition-major so the
    DMA is one contiguous 2D copy."""
    nch = a.shape[0] // P
    kcf = a.reshape(nch, P, -1)  # [chunk, p, F]
    return np.ascontiguousarray(np.transpose(kcf, (1, 0, 2)).reshape(P, -1))


def _host_prep(x, cos, sin, w_qkv, w_proj):
    """Build the 8 per-core input maps (everything pre-packed to the SBUF
    partition-major layout)."""
    f16 = np.float16

    # x: [C, T] kc-major rows -> [128, nb-major (kc, t-block)] columns
    xbs = []
    for b in range(B):
        xT = x[b].T.reshape(KC, P, NQB, TQ)  # [kc, p, nb, t]
        xb = np.transpose(xT, (1, 2, 0, 3)).reshape(P, -1)  # [p, nb*kc*TQ]
        xbs.append(np.ascontiguousarray(xb).astype(f16))

    cosT = np.ascontiguousarray(cos.T)  # [32, T]
    sinT = np.ascontiguousarray(sin.T)
    cc64 = np.concatenate([cosT[0:16], cosT[0:16], cosT[16:32], cosT[16:32]])
    ss64 = np.concatenate([-sinT[0:16], sinT[0:16], -sinT[16:32], sinT[16:32]])
    cc = np.tile(cc64, (2, 1)).astype(f16)  # [128, T]
    ss = np.tile(ss64, (2, 1)).astype(f16)

    # ebc[k, p*128+m] = 1 where k == 32*(2p + m//64): broadcasts den row
    ebc = np.zeros((P, 2 * P), dtype=np.float32)
    for p in range(2):
        for i in range(2):
            ebc[32 * (2 * p + i), p * P + i * HD : p * P + (i + 1) * HD] = 1.0

    wq = w_qkv[0:C]
    wk = w_qkv[C : 2 * C]
    wv = w_qkv[2 * C : 3 * C]

    in_maps = []
    for core in range(N_CORES):
        b = core // 4
        h0 = 4 * (core % 4)
        heads = [h0, h0 + 1, h0 + 2, h0 + 3]

        def pair_block(w, pair):
            ha, hb = heads[2 * pair], heads[2 * pair + 1]
            return np.concatenate(
                [_eo(w[ha * HD : ha * HD + HD]), _eo(w[hb * HD : hb * HD + HD])],
                axis=0,
            )  # [128, C]

        # per-kc [128, 2P] blocks: cols = [pair0 | pair1] m-offsets
        wq2 = np.concatenate([pair_block(wq, 0), pair_block(wq, 1)], axis=0)  # [256,C]
        wk2 = np.concatenate([pair_block(wk, 0), pair_block(wk, 1)], axis=0)
        # rows are m (proj channel), cols C (contraction): want [128p, kc*2P]
        # with lhsT[p=contraction chunk rows, m]: lhsT col-block kc holds
        # w[m, kc*128+p] at [p, kc*256 + m]
        wqkq = np.ascontiguousarray(
            np.transpose(wq2.T.reshape(KC, P, 2 * P), (1, 0, 2)).reshape(P, -1)
        ).astype(f16)
        wqkk = np.ascontiguousarray(
            np.transpose(wk2.T.reshape(KC, P, 2 * P), (1, 0, 2)).reshape(P, -1)
        ).astype(f16)
        wvp = _pack_rows(
            np.ascontiguousarray(wv[h0 * HD : h0 * HD + GC].T)  # [C, 256]
        ).astype(f16)
        wpp = _pack_rows(
            np.ascontiguousarray(w_proj[:, h0 * HD : h0 * HD + GC].T)  # [256, C]
        ).astype(f16)
        in_maps.append(
            {
                "xb": xbs[b],
                "wqkk": wqkk,
                "wqkq": wqkq,
                "wvp": wvp,
                "wpp": wpp,
                "cc": cc,
                "ss": ss,
                "ebc": ebc,
            }
        )
    return in_maps


def kernel(x, cos, sin, mask, w_qkv, w_proj, _trace=False, _tmpdir=None):
    x = np.asarray(x, dtype=np.float32)
    cos = np.asarray(cos, dtype=np.float32)
    sin = np.asarray(sin, dtype=np.float32)
    w_qkv = np.asarray(w_qkv, dtype=np.float32)
    w_proj = np.asarray(w_proj, dtype=np.float32)
    # mask is all-ones in this problem spec: no-op in the math.

    nc = _get_program()
    in_maps = _host_prep(x, cos, sin, w_qkv, w_proj)
    res = run_bass_kernel_spmd(
        nc, in_maps, list(range(N_CORES)), trace=_trace, tmpdir=_tmpdir
    )
    out = np.empty((B, T, C), dtype=np.float32)
    for b in range(B):
        acc = res.results[4 * b]["y"].astype(np.float32).copy()
        for g in range(1, 4):
            acc += res.results[4 * b + g]["y"]
        out[b] = acc
    kernel._last_exec_time_ns = res.exec_time_ns
    return out
